# revision 6
# baseline (speedup 1.0000x reference)
"""Grouped-expert SwiGLU MLP (MoE grouped GEMM) on 8 Trainium2 NeuronCores.

Strategy: expert-parallel SPMD. Core e owns expert e's weights and its
contiguous token block (tokens are pre-grouped by expert). All experts are
host-padded to a uniform tile count so a single SPMD program serves all 8
cores; per-core divergence lives entirely in the input data.

Per 512-token M-tile (on-chip, per core):
  xT   [128d x (16k x 512t)]   transposed load of x slice
  X1T_f/X3T_f (PSUM)[128f x 512t] = sum_k w{1,3}[k,f-chunk].T @ xT_k   (float32r)
  hT_f (SBUF) = silu(X1T_f) * X3T_f
  out[ts,dc] (PSUM)[128t x 512d] = sum_fc hT[fc,ts].T @ w2[fc,dc]      (float32r)
  out -> DRAM contiguous rows.
"""

import math
import os

import numpy as np

D = 2048          # model dim
F = 512           # expert ffn dim
MT = 512          # tokens per M-tile
KC = D // 128     # 16 contraction chunks of 128
FC = F // 128     # 4 f chunks of 128
NCORES = 8

_cache = {}


def _build(pad_tiles: int):
    """Build + schedule the single-core SPMD program for pad_tiles M-tiles."""
    import concourse.bacc as bacc
    import concourse.mybir as mybir
    from concourse.tile import TileContext

    dt = mybir.dt
    f32 = dt.float32
    f32r = dt.float32r
    PAD_T = pad_tiles * MT

    nc = bacc.Bacc(
        "TRN2",
        target_bir_lowering=False,
        debug=False,
        enable_asserts=False,
        num_devices=NCORES,
    )

    xp = nc.dram_tensor("xp", [PAD_T, D], f32, kind="ExternalInput")
    w1 = nc.dram_tensor("w1", [D, F], f32, kind="ExternalInput")
    w2 = nc.dram_tensor("w2", [F, D], f32, kind="ExternalInput")
    w3 = nc.dram_tensor("w3", [D, F], f32, kind="ExternalInput")
    out = nc.dram_tensor("out", [PAD_T, D], f32, kind="ExternalOutput")

    with TileContext(nc) as tc:
        with (
            tc.tile_pool(name="wpool", bufs=1) as wpool,
            tc.tile_pool(name="xt", bufs=6) as xt_pool,
            tc.tile_pool(name="ht", bufs=2) as ht_pool,
            tc.tile_pool(name="sil", bufs=3) as sil_pool,
            tc.tile_pool(name="osb", bufs=3) as osb_pool,
            tc.tile_pool(name="ps", bufs=8, space="PSUM") as ps_pool,
        ):
            # --- resident weights ---------------------------------------
            # w1/w3 stored [128d x (k,f)] : chunk (k) occupies free cols
            # [k*F, (k+1)*F); within it f-chunk at f*128.
            w1_sb = wpool.tile([128, KC, F], f32r, tag="w1")
            w3_sb = wpool.tile([128, KC, F], f32r, tag="w3")
            # w2 stored [128f x (fc,d)]: fc chunk at free cols [fc*D,(fc+1)*D)
            w2_sb = wpool.tile([128, FC, D], f32r, tag="w2")

            nc.sync.dma_start(
                out=w1_sb[:], in_=w1.ap().rearrange("(k p) f -> p k f", p=128).bitcast(f32r)
            )
            nc.sync.dma_start(
                out=w3_sb[:], in_=w3.ap().rearrange("(k p) f -> p k f", p=128).bitcast(f32r)
            )
            nc.sync.dma_start(
                out=w2_sb[:], in_=w2.ap().rearrange("(c p) d -> p c d", p=128).bitcast(f32r)
            )

            for m in range(pad_tiles):
                # --- transposed x load: 4 tiles of 4 k-chunks each ------
                xts = []
                for g in range(4):
                    xt = xt_pool.tile([128, 4, MT], f32r, tag="xt")
                    for kk in range(4):
                        k = g * 4 + kk
                        nc.sync.dma_start(
                            out=xt[:, kk, :],
                            in_=xp[m * MT:(m + 1) * MT, k * 128:(k + 1) * 128]
                            .rearrange("t d -> d t").bitcast(f32r),
                        )
                    xts.append(xt)

                # --- GEMM1 + GEMM3 + SwiGLU, per f-chunk ----------------
                ht = ht_pool.tile([128, FC, MT], f32r, tag="ht")
                for f in range(FC):
                    x1t = ps_pool.tile([128, MT], f32, tag="ps")
                    x3t = ps_pool.tile([128, MT], f32, tag="ps")
                    for k in range(KC):
                        lhs1 = w1_sb[:, k, f * 128:(f + 1) * 128]
                        lhs3 = w3_sb[:, k, f * 128:(f + 1) * 128]
                        rhs = xts[k // 4][:, k % 4, :]
                        nc.tensor.matmul(
                            x1t[:], lhs1, rhs,
                            start=(k == 0), stop=(k == KC - 1),
                        )
                        nc.tensor.matmul(
                            x3t[:], lhs3, rhs,
                            start=(k == 0), stop=(k == KC - 1),
                        )
                    sig = sil_pool.tile([128, MT], f32, tag="sig")
                    nc.scalar.activation(
                        sig[:], x1t[:], mybir.ActivationFunctionType.Sigmoid
                    )
                    sil = sil_pool.tile([128, MT], f32, tag="sil")
                    nc.vector.tensor_mul(sil[:], x1t[:], sig[:])
                    nc.vector.tensor_mul(
                        ht[:, f, :], sil[:], x3t[:]
                    )

                # --- GEMM2: out[ts,dc] = sum_fc hT[fc,ts].T @ w2[fc,dc] -
                for ts in range(4):
                    osb = osb_pool.tile([128, D], f32, tag="osb")
                    for dc in range(4):
                        po = ps_pool.tile([128, 512], f32, tag="ps")
                        for fc in range(FC):
                            lhs = ht[:, fc, ts * 128:(ts + 1) * 128]
                            rhs = w2_sb[:, fc, dc * 512:(dc + 1) * 512]
                            nc.tensor.matmul(
                                po[:], lhs, rhs,
                                start=(fc == 0), stop=(fc == FC - 1),
                            )
                        nc.vector.tensor_copy(osb[:, dc * 512:(dc + 1) * 512], po[:])
                    nc.sync.dma_start(
                        out=out[m * MT + ts * 128: m * MT + (ts + 1) * 128, :],
                        in_=osb[:],
                    )

    nc.compile()
    return nc


def _get_program(pad_tiles: int):
    if pad_tiles not in _cache:
        _cache[pad_tiles] = _build(pad_tiles)
    return _cache[pad_tiles]


def kernel(x, num_tokens_per_expert, w1, w2, w3):
    from concourse.bass_utils import run_bass_kernel_spmd

    x = np.asarray(x)
    counts = np.asarray(num_tokens_per_expert).astype(np.int64)
    w1 = np.asarray(w1)
    w2 = np.asarray(w2)
    w3 = np.asarray(w3)

    T = x.shape[0]
    E = counts.shape[0]
    assert E == NCORES, f"expected {NCORES} experts, got {E}"
    starts = np.concatenate([[0], np.cumsum(counts)])[:E]

    pad_tiles = max(1, math.ceil(int(counts.max()) / MT))
    nc = _get_program(pad_tiles)
    PAD_T = pad_tiles * MT

    in_maps = []
    for e in range(E):
        cnt = int(counts[e])
        s = int(starts[e])
        xp = np.zeros((PAD_T, D), dtype=np.float32)
        xp[:cnt] = x[s:s + cnt]
        in_maps.append({
            "xp": xp,
            "w1": np.ascontiguousarray(w1[e]),
            "w2": np.ascontiguousarray(w2[e]),
            "w3": np.ascontiguousarray(w3[e]),
        })

    trace = bool(int(os.environ.get("KERNEL_TRACE", "0")))
    try:
        res = run_bass_kernel_spmd(
            nc, in_maps, core_ids=list(range(NCORES)), trace=trace
        )
    except ModuleNotFoundError:
        res = run_bass_kernel_spmd(
            nc, in_maps, core_ids=list(range(NCORES)), trace=False
        )
    kernel.last_results = res

    out = np.empty((T, D), dtype=np.float32)
    for e in range(E):
        cnt = int(counts[e])
        s = int(starts[e])
        out[s:s + cnt] = res.results[e]["out"][:cnt]
    return out


# revision 7
# speedup vs baseline: 1.0317x; 1.0317x over previous
"""Grouped-expert SwiGLU MLP (MoE grouped GEMM) on 8 Trainium2 NeuronCores.

Strategy: expert-parallel SPMD. Core e owns expert e's weights and its
contiguous token block (tokens are pre-grouped by expert). All experts are
host-padded to a uniform tile count so a single SPMD program serves all 8
cores; per-core divergence lives entirely in the input data.

Per 512-token M-tile (on-chip, per core):
  xT   [128d x (16k x 512t)]   transposed load of x slice
  X1T_f/X3T_f (PSUM)[128f x 512t] = sum_k w{1,3}[k,f-chunk].T @ xT_k   (float32r)
  hT_f (SBUF) = silu(X1T_f) * X3T_f
  out[ts,dc] (PSUM)[128t x 512d] = sum_fc hT[fc,ts].T @ w2[fc,dc]      (float32r)
  out -> DRAM contiguous rows.
"""

import math
import os

import numpy as np

D = 2048          # model dim
F = 512           # expert ffn dim
MT = 512          # tokens per M-tile
KC = D // 128     # 16 contraction chunks of 128
FC = F // 128     # 4 f chunks of 128
NCORES = 8

_cache = {}


def _build(pad_tiles: int):
    """Build + schedule the single-core SPMD program for pad_tiles M-tiles."""
    import concourse.bacc as bacc
    import concourse.mybir as mybir
    from concourse.tile import TileContext

    dt = mybir.dt
    f32 = dt.float32
    f32r = dt.float32r
    PAD_T = pad_tiles * MT

    nc = bacc.Bacc(
        "TRN2",
        target_bir_lowering=False,
        debug=False,
        enable_asserts=False,
        num_devices=NCORES,
    )

    xpt = nc.dram_tensor("xpt", [D, PAD_T], f32, kind="ExternalInput")
    w1 = nc.dram_tensor("w1", [D, F], f32, kind="ExternalInput")
    w2 = nc.dram_tensor("w2", [F, D], f32, kind="ExternalInput")
    w3 = nc.dram_tensor("w3", [D, F], f32, kind="ExternalInput")
    out = nc.dram_tensor("out", [PAD_T, D], f32, kind="ExternalOutput")

    with TileContext(nc) as tc:
        with (
            tc.tile_pool(name="wpool", bufs=1) as wpool,
            tc.tile_pool(name="xt", bufs=6) as xt_pool,
            tc.tile_pool(name="ht", bufs=2) as ht_pool,
            tc.tile_pool(name="sil", bufs=3) as sil_pool,
            tc.tile_pool(name="osb", bufs=3) as osb_pool,
            tc.tile_pool(name="ps", bufs=8, space="PSUM") as ps_pool,
        ):
            # --- resident weights ---------------------------------------
            # w1/w3 stored [128d x (k,f)] : chunk (k) occupies free cols
            # [k*F, (k+1)*F); within it f-chunk at f*128.
            w1_sb = wpool.tile([128, KC, F], f32r, tag="w1")
            w3_sb = wpool.tile([128, KC, F], f32r, tag="w3")
            # w2 stored [128f x (fc,d)]: fc chunk at free cols [fc*D,(fc+1)*D)
            w2_sb = wpool.tile([128, FC, D], f32r, tag="w2")

            nc.sync.dma_start(
                out=w1_sb[:], in_=w1.ap().rearrange("(k p) f -> p k f", p=128).bitcast(f32r)
            )
            nc.sync.dma_start(
                out=w3_sb[:], in_=w3.ap().rearrange("(k p) f -> p k f", p=128).bitcast(f32r)
            )
            nc.sync.dma_start(
                out=w2_sb[:], in_=w2.ap().rearrange("(c p) d -> p c d", p=128).bitcast(f32r)
            )

            for m in range(pad_tiles):
                # --- transposed x load: 4 tiles of 4 k-chunks each ------
                xts = []
                for g in range(4):
                    xt = xt_pool.tile([128, 4, MT], f32r, tag="xt")
                    for kk in range(4):
                        k = g * 4 + kk
                        nc.sync.dma_start(
                            out=xt[:, kk, :],
                            in_=xpt[k * 128:(k + 1) * 128, m * MT:(m + 1) * MT]
                            .bitcast(f32r),
                        )
                    xts.append(xt)

                # --- GEMM1 + GEMM3 + SwiGLU, per f-chunk ----------------
                ht = ht_pool.tile([128, FC, MT], f32r, tag="ht")
                for f in range(FC):
                    x1t = ps_pool.tile([128, MT], f32, tag="ps")
                    x3t = ps_pool.tile([128, MT], f32, tag="ps")
                    for k in range(KC):
                        lhs1 = w1_sb[:, k, f * 128:(f + 1) * 128]
                        lhs3 = w3_sb[:, k, f * 128:(f + 1) * 128]
                        rhs = xts[k // 4][:, k % 4, :]
                        nc.tensor.matmul(
                            x1t[:], lhs1, rhs,
                            start=(k == 0), stop=(k == KC - 1),
                        )
                        nc.tensor.matmul(
                            x3t[:], lhs3, rhs,
                            start=(k == 0), stop=(k == KC - 1),
                        )
                    sig = sil_pool.tile([128, MT], f32, tag="sig")
                    nc.scalar.activation(
                        sig[:], x1t[:], mybir.ActivationFunctionType.Sigmoid
                    )
                    sil = sil_pool.tile([128, MT], f32, tag="sil")
                    nc.vector.tensor_mul(sil[:], x1t[:], sig[:])
                    nc.vector.tensor_mul(
                        ht[:, f, :], sil[:], x3t[:]
                    )

                # --- GEMM2: out[ts,dc] = sum_fc hT[fc,ts].T @ w2[fc,dc] -
                for ts in range(4):
                    osb = osb_pool.tile([128, D], f32, tag="osb")
                    for dc in range(4):
                        po = ps_pool.tile([128, 512], f32, tag="ps")
                        for fc in range(FC):
                            lhs = ht[:, fc, ts * 128:(ts + 1) * 128]
                            rhs = w2_sb[:, fc, dc * 512:(dc + 1) * 512]
                            nc.tensor.matmul(
                                po[:], lhs, rhs,
                                start=(fc == 0), stop=(fc == FC - 1),
                            )
                        nc.vector.tensor_copy(osb[:, dc * 512:(dc + 1) * 512], po[:])
                    nc.sync.dma_start(
                        out=out[m * MT + ts * 128: m * MT + (ts + 1) * 128, :],
                        in_=osb[:],
                    )

    nc.compile()
    return nc


def _get_program(pad_tiles: int):
    if pad_tiles not in _cache:
        _cache[pad_tiles] = _build(pad_tiles)
    return _cache[pad_tiles]


def kernel(x, num_tokens_per_expert, w1, w2, w3):
    from concourse.bass_utils import run_bass_kernel_spmd

    x = np.asarray(x)
    counts = np.asarray(num_tokens_per_expert).astype(np.int64)
    w1 = np.asarray(w1)
    w2 = np.asarray(w2)
    w3 = np.asarray(w3)

    T = x.shape[0]
    E = counts.shape[0]
    assert E == NCORES, f"expected {NCORES} experts, got {E}"
    starts = np.concatenate([[0], np.cumsum(counts)])[:E]

    pad_tiles = max(1, math.ceil(int(counts.max()) / MT))
    nc = _get_program(pad_tiles)
    PAD_T = pad_tiles * MT

    in_maps = []
    for e in range(E):
        cnt = int(counts[e])
        s = int(starts[e])
        xpt = np.zeros((D, PAD_T), dtype=np.float32)
        xpt[:, :cnt] = x[s:s + cnt].T
        in_maps.append({
            "xpt": xpt,
            "w1": np.ascontiguousarray(w1[e]),
            "w2": np.ascontiguousarray(w2[e]),
            "w3": np.ascontiguousarray(w3[e]),
        })

    trace = bool(int(os.environ.get("KERNEL_TRACE", "0")))
    try:
        res = run_bass_kernel_spmd(
            nc, in_maps, core_ids=list(range(NCORES)), trace=trace
        )
    except ModuleNotFoundError:
        res = run_bass_kernel_spmd(
            nc, in_maps, core_ids=list(range(NCORES)), trace=False
        )
    kernel.last_results = res

    out = np.empty((T, D), dtype=np.float32)
    for e in range(E):
        cnt = int(counts[e])
        s = int(starts[e])
        out[s:s + cnt] = res.results[e]["out"][:cnt]
    return out
